# revision 1
# baseline (speedup 1.0000x reference)
"""Trainium2 Bass kernel for nn_ConvLayer_56453050139435.

Reference computation (StyleGAN2-style downsampling conv layer):
  1. depthwise 4x4 binomial blur ([1,3,3,1] outer [1,3,3,1] / 64) with pad 2
  2. 3x3 stride-2 conv, 128 -> 256 channels, weight scaled by 1/sqrt(fan_in)
  3. bias + leaky-relu(0.2) * sqrt(2), clamp +-256 (never binds: |out| < ~4)

Sharding: data-parallel over batch, 2 images per core across 8 cores.

Per-core pipeline (fp16 data path, fp32 PSUM accumulation), fully
block-streamed: each block of 8 output rows owns a 20-row strip of the
input (4-row halo recomputed between blocks) so DMA / ACT / DVE / PE
pipeline at block granularity with small pool-rotated tiles:
  - horizontal blur on DVE (2 shifted adds + scalar_tensor_tensor), using an
    ACT-engine shifted copy so every DVE operand is 4-byte aligned (2x mode)
  - vertical blur on DVE (aligned row-pitch shifts)
  - conv as 9-tap matmul accumulation in PSUM, oc split in two 128-halves,
    rhs = stride-2 access pattern on the blurred strip
  - epilogue: one ACT Prelu op (scale=sqrt2, bias, alpha=0.2) PSUM->SBUF fp16
  - DMA out fp16, host-cast back to fp32
"""

import numpy as np

import concourse.bass as bass
import concourse.mybir as mybir
from concourse import bacc
from concourse.tile import TileContext
from concourse.bass_utils import run_bass_kernel_spmd

AF = mybir.ActivationFunctionType
OP = mybir.AluOpType
FP16 = mybir.dt.float16
FP32 = mybir.dt.float32

IC, OC, H, W = 128, 256, 256, 256
OH, OW = 128, 128
KS = 3
N_CORES = 8
B_PER_CORE = 2
SQRT2 = float(np.sqrt(2.0))
WSCALE = 1.0 / float(np.sqrt(KS * KS * IC))
LRELU_SLOPE = 0.2

XPITCH = 264     # padded x row pitch: x col w lives at buffer col w+2
HB_W = 260       # blur cols 0..256 valid, 257+ garbage (never consumed)
SROWS = 36       # hb rows per pair of blocks (32 vb rows + 4-row halo)
NPAIR = 8        # pairs of 8-output-row blocks per image


def _build_nc():
    nc = bacc.Bacc(None, target_bir_lowering=False)
    x_d = nc.dram_tensor("x", [B_PER_CORE, IC, H, W], FP16, kind="ExternalInput")
    w_d = nc.dram_tensor("w", [IC, 18 * 128], FP16, kind="ExternalInput")
    b_d = nc.dram_tensor("b", [128, 2], FP32, kind="ExternalInput")
    y_d = nc.dram_tensor("y", [B_PER_CORE, OC, OH, OW], FP16, kind="ExternalOutput")

    with TileContext(nc) as tc:
        with (
            tc.tile_pool(name="const", bufs=1) as cpool,
            tc.tile_pool(name="xin", bufs=2) as xpool,
            tc.tile_pool(name="shift", bufs=1) as spool,
            tc.tile_pool(name="scr", bufs=1) as scrpool,
            tc.tile_pool(name="hb", bufs=1) as hbpool,
            tc.tile_pool(name="vb", bufs=3) as vbpool,
            tc.tile_pool(name="out", bufs=4) as opool,
            tc.tile_pool(name="psum", bufs=8, space="PSUM") as pspool,
        ):
            wt = cpool.tile([128, 18 * 128], FP16)
            bt = cpool.tile([128, 2], FP32)
            al = cpool.tile([128, 1], FP32)
            nc.sync.dma_start(wt[:], w_d[:])
            nc.sync.dma_start(bt[:], b_d[:])
            nc.vector.memset(al[:], LRELU_SLOPE)

            # DVE-only scratch: single-buffered (engine order serializes)
            t1 = scrpool.tile([128, SROWS, HB_W], FP16)
            t2 = scrpool.tile([128, SROWS, HB_W], FP16)
            t3 = scrpool.tile([128, SROWS, HB_W], FP16)
            vu = scrpool.tile([128, 17, HB_W], FP16)
            vw = scrpool.tile([128, 17, HB_W], FP16)

            for img in range(B_PER_CORE):
                for P in range(NPAIR):
                    lo = 32 * P - 2       # x row of hb tile row 0
                    xr0 = max(lo, 0)
                    xr1 = min(lo + SROWS, H)
                    ta, tb = xr0 - lo, xr1 - lo  # valid hb tile row range

                    xt = xpool.tile([128, SROWS, XPITCH], FP16)
                    # x cols -2..-1 and 256..257 must be zero; cols beyond are
                    # garbage that only feeds hb cols >256 (never consumed)
                    nc.gpsimd.memset(xt[:, ta:tb, 0:2], 0.0)
                    nc.gpsimd.memset(xt[:, ta:tb, 258:260], 0.0)
                    nc.sync.dma_start(
                        xt[:, ta:tb, 2:258], x_d[img, :, xr0:xr1, :]
                    )
                    # shifted copy (ACT): ct[j] = xt[j+1], keeps DVE aligned
                    ct = spool.tile([128, SROWS, 262], FP16)
                    nc.scalar.copy(ct[:, ta:tb, :], xt[:, ta:tb, 1:263])

                    hb = hbpool.tile([128, SROWS, HB_W], FP16)
                    if ta > 0:
                        nc.gpsimd.memset(hb[:, 0:ta, :], 0.0)
                    if tb < SROWS:
                        nc.gpsimd.memset(hb[:, tb:SROWS, :], 0.0)
                    # hb[c] = x[c-2] + 3x[c-1] + 3x[c] + x[c+1]
                    #       = (xt[c] + ct[c+2]) + 3*(ct[c] + xt[c+2])
                    nc.vector.tensor_tensor(
                        out=t1[:, ta:tb, :], in0=xt[:, ta:tb, 0:HB_W],
                        in1=ct[:, ta:tb, 2:262], op=OP.add,
                    )
                    nc.vector.tensor_tensor(
                        out=t2[:, ta:tb, :], in0=ct[:, ta:tb, 0:HB_W],
                        in1=xt[:, ta:tb, 2 : 2 + HB_W], op=OP.add,
                    )
                    nc.vector.tensor_scalar_mul(t3[:, ta:tb, :], t2[:, ta:tb, :], 3.0)
                    nc.vector.tensor_tensor(
                        out=hb[:, ta:tb, :], in0=t1[:, ta:tb, :],
                        in1=t3[:, ta:tb, :], op=OP.add,
                    )
                    # t3 <- 3*hb (full 36 rows: edge rows of hb are zeroed)
                    nc.vector.tensor_scalar_mul(t3[:], hb[:], 3.0)

                    for s in range(2):
                        rbl = 16 * s
                        p0 = 16 * P + 8 * s   # first output row of sub-block
                        # vertical blur:
                        # vb[v] = (hb[v] + 3hb[v+1]) + (3hb[v+2] + hb[v+3])
                        vb = vbpool.tile([128, 17, HB_W], FP16)
                        nc.vector.tensor_tensor(
                            out=vu[:], in0=hb[:, rbl : rbl + 17, :],
                            in1=t3[:, rbl + 1 : rbl + 18, :], op=OP.add,
                        )
                        nc.vector.tensor_tensor(
                            out=vw[:], in0=t3[:, rbl + 2 : rbl + 19, :],
                            in1=hb[:, rbl + 3 : rbl + 20, :], op=OP.add,
                        )
                        nc.vector.tensor_tensor(
                            out=vb[:], in0=vu[:], in1=vw[:], op=OP.add,
                        )

                        for oc_h in range(2):
                            ot = opool.tile([128, 8, OW], FP16)
                            for ch in range(2):
                                ps = pspool.tile([128, 4, OW], FP32)
                                for t in range(9):
                                    kh, kw = t // 3, t % 3
                                    idx = t * 2 + oc_h
                                    nc.tensor.matmul(
                                        ps[:],
                                        wt[:, idx * 128 : (idx + 1) * 128],
                                        vb[:, 8 * ch + kh : 8 * ch + kh + 7 : 2,
                                           kw : kw + 255 : 2],
                                        start=(t == 0),
                                        stop=(t == 8),
                                    )
                                nc.scalar.activation(
                                    ot[:, 4 * ch : 4 * ch + 4, :], ps[:],
                                    AF.Prelu,
                                    bias=bt[:, oc_h : oc_h + 1],
                                    scale=SQRT2,
                                    alpha=al[:, 0:1],
                                )
                            nc.sync.dma_start(
                                y_d[img, 128 * oc_h : 128 * (oc_h + 1),
                                    p0 : p0 + 8, :],
                                ot[:],
                            )
    nc.finalize()
    return nc


_NC = None


def _get_nc():
    global _NC
    if _NC is None:
        _NC = _build_nc()
    return _NC


def kernel(x, weight, bias):
    x = np.asarray(x, dtype=np.float32)
    weight = np.asarray(weight, dtype=np.float32)
    bias = np.asarray(bias, dtype=np.float32)

    # host-side prep: fold wscale and the blur's 1/64 norm into the weights,
    # sqrt(2) gain and lr_mul into the bias; lay out lhsT tiles per (tap, half)
    w_eff = (weight * (WSCALE / 64.0)).astype(np.float16)  # [256,128,3,3]
    w_sb = np.empty((IC, 18 * 128), dtype=np.float16)
    for t in range(9):
        kh, kw = t // 3, t % 3
        for oc_h in range(2):
            idx = t * 2 + oc_h
            w_sb[:, idx * 128 : (idx + 1) * 128] = (
                w_eff[oc_h * 128 : (oc_h + 1) * 128, :, kh, kw].T
            )
    b_sb = (SQRT2 * bias).astype(np.float32).reshape(2, 128).T.copy()  # [128,2]

    x16 = x.astype(np.float16)
    nc = _get_nc()
    in_maps = [
        {
            "x": x16[c * B_PER_CORE : (c + 1) * B_PER_CORE],
            "w": w_sb,
            "b": b_sb,
        }
        for c in range(N_CORES)
    ]
    res = run_bass_kernel_spmd(nc, in_maps, core_ids=list(range(N_CORES)))
    y16 = np.concatenate([res.results[c]["y"] for c in range(N_CORES)], axis=0)
    return y16.astype(np.float32)

